# revision 10
# baseline (speedup 1.0000x reference)
"""DWA LanguageModel layer on 8 trn2 NeuronCores.

Strategy:
  - Tokens (B=1024) data-parallel across 8 cores (128 each).
  - Keys (pool @ W_K, token-independent) N-sharded: each core computes
    normalized+aspect-weighted keys for its 128 pool rows, AllGather.
  - Low-rank update never materializes UV: h_delta[b] = sum_n alpha[b,n]
    U_n (V_n z_b) via dense matmuls over nr=(n,r) with alpha sparsity
    handled by top-16 masking (masked alpha entries are exactly 0).
  - Top-16 threshold per token via vector.max + match_replace + vector.max.
  - alpha normalization deferred to the final combine (linear in alpha).
"""
import sys

sys.path.insert(0, "/opt/trn_rl_repo")
import numpy as np
import ml_dtypes

import concourse.bass as bass
import concourse.mybir as mybir
import concourse.tile as tile
from concourse import bacc
from concourse.bass_utils import run_bass_kernel_spmd
from concourse.masks import make_identity

F32 = mybir.dt.float32
BF16 = mybir.dt.bfloat16
AF = mybir.ActivationFunctionType
ALU = mybir.AluOpType

NCORES = 8
B = 1024            # tokens
BL = B // NCORES    # tokens per core = 128
D_MODEL = 512
N = 1024            # pool rows
D = 16384           # pool cols
S = 2
DK = 64
SDK = S * DK        # 128
R = 8
NR = N * R          # 8192
K_MAX = 16
LAMBDA_SHARP = 5.0
LN_EPS = 1e-5
U_END = D_MODEL * R          # 4096
V_END = U_END + R * D_MODEL  # 8192
B_END = V_END + D_MODEL      # 8704
NL = N // NCORES             # pool rows per core = 128

# dtype config: keys path and dynamic (post-alpha) path
DT_KEYS = F32
DT_DYN = F32

LAST_EXEC_NS = None
TRACE = False
LEVEL = 9  # bisect: 0=io 1=keys 2=+AG 3=+scores 4=alpha 5=s 6=no-bias 9=full


def _np_dt(dt):
    return {F32: np.float32, BF16: ml_dtypes.bfloat16}[dt]


def _build(tau_f, w0_f, w1_f, gamma_f):
    nc = bacc.Bacc("TRN2", target_bir_lowering=False, debug=False,
                   num_devices=NCORES)

    # ---- I/O ----
    z_d = nc.dram_tensor("z", [BL, D_MODEL], F32, kind="ExternalInput")
    zt_d = nc.dram_tensor("zt", [D_MODEL, BL], F32, kind="ExternalInput")
    poolT_d = nc.dram_tensor("poolT", [D, NL], DT_KEYS, kind="ExternalInput")
    wk_d = nc.dram_tensor("wk", [D, SDK], DT_KEYS, kind="ExternalInput")
    wq_d = nc.dram_tensor("wq", [D_MODEL, SDK], F32, kind="ExternalInput")
    vt_d = nc.dram_tensor("vt", [D_MODEL, NR], DT_DYN, kind="ExternalInput")
    up_d = nc.dram_tensor("up", [NR, D_MODEL], DT_DYN, kind="ExternalInput")
    pb_d = nc.dram_tensor("pb", [N, D_MODEL], DT_DYN, kind="ExternalInput")
    wbt_d = nc.dram_tensor("wbt", [D_MODEL, D_MODEL], F32, kind="ExternalInput")
    gb_d = nc.dram_tensor("gb", [BL, D_MODEL], F32, kind="ExternalInput")
    ls_d = nc.dram_tensor("ls", [BL, D_MODEL], F32, kind="ExternalInput")
    lb_d = nc.dram_tensor("lb", [BL, D_MODEL], F32, kind="ExternalInput")
    out_d = nc.dram_tensor("out", [BL, D_MODEL], F32, kind="ExternalOutput")

    with tile.TileContext(nc) as tc:
        with (
            tc.tile_pool(name="sb", bufs=1) as sb,            # persistent tiles
            tc.tile_pool(name="sbs", bufs=4) as sbs,          # streamed tiles
            tc.tile_pool(name="dram", bufs=1, space="DRAM") as dram,
        ):
            _emit(nc, tc, sb, sbs, dram, locals(), tau_f, w0_f, w1_f, gamma_f,
                  z_d, zt_d, poolT_d, wk_d, wq_d, vt_d, up_d, pb_d, wbt_d,
                  gb_d, ls_d, lb_d, out_d)

    nc.compile()
    return nc


def _emit(nc, tc, sb, sbs, dram, _l, tau_f, w0_f, w1_f, gamma_f,
          z_d, zt_d, poolT_d, wk_d, wq_d, vt_d, up_d, pb_d, wbt_d,
          gb_d, ls_d, lb_d, out_d):
    if LEVEL <= 0:
        z0 = sb.tile([BL, D_MODEL], F32, tag="z0")
        nc.sync.dma_start(z0[:], z_d[:])
        pt0 = sb.tile([128, NL], DT_KEYS, tag="pt0")
        nc.sync.dma_start(pt0[:], poolT_d[0:128, :])
        vt0 = sb.tile([128, 512], DT_DYN, tag="vt0")
        nc.sync.dma_start(vt0[:], vt_d[0:128, 0:512])
        up0 = sb.tile([128, 512], DT_DYN, tag="up0")
        nc.sync.dma_start(up0[:], up_d[0:128, :])
        o0 = sb.tile([BL, D_MODEL], F32, tag="o0")
        nc.vector.tensor_scalar_mul(o0[:], z0[:], 2.0)
        nc.sync.dma_start(out_d[:], o0[:])
        return

    # ---------- small persistent loads ----------
    ident = sb.tile([128, 128], F32, tag="ident")
    make_identity(nc, ident[:])
    z_sb = sb.tile([BL, D_MODEL], F32, tag="z")
    nc.sync.dma_start(z_sb[:], z_d[:])
    gb_sb = sb.tile([BL, D_MODEL], F32, tag="gb")
    nc.sync.dma_start(gb_sb[:], gb_d[:])
    ls_sb = sb.tile([BL, D_MODEL], F32, tag="ls")
    nc.sync.dma_start(ls_sb[:], ls_d[:])
    lb_sb = sb.tile([BL, D_MODEL], F32, tag="lb")
    nc.sync.dma_start(lb_sb[:], lb_d[:])
    # zb = z + gamma * b_base   (gb = gamma*b_base replicated)
    zb_sb = sb.tile([BL, D_MODEL], F32, tag="zb")
    nc.vector.tensor_add(zb_sb[:], z_sb[:], gb_sb[:])

    # zt: [128, 4*128], chunk c holds rows a=c*128..(c+1)*128 of z^T
    zt_sb = sb.tile([128, D_MODEL], F32, tag="zt")
    for c in range(4):
        nc.sync.dma_start(zt_sb[:, c * 128:(c + 1) * 128],
                          zt_d[c * 128:(c + 1) * 128, :])
    wq_sb = sb.tile([128, 4 * SDK], F32, tag="wq")
    for c in range(4):
        nc.sync.dma_start(wq_sb[:, c * SDK:(c + 1) * SDK],
                          wq_d[c * 128:(c + 1) * 128, :])
    if DT_DYN != F32:
        ztd_sb = sb.tile([128, D_MODEL], DT_DYN, tag="ztd")
        nc.vector.tensor_copy(ztd_sb[:], zt_sb[:])
    else:
        ztd_sb = zt_sb

    # ---------- phase A: keys (N-sharded) ----------
    with tc.tile_pool(name="psA", bufs=1, space="PSUM") as psA:
        keys_ps = psA.tile([NL, SDK], F32, tag="keys")
        nch = D // 128
        for d in range(nch):
            pt = sbs.tile([128, NL], DT_KEYS, tag="pt")
            nc.sync.dma_start(pt[:], poolT_d[d * 128:(d + 1) * 128, :])
            wkt = sbs.tile([128, SDK], DT_KEYS, tag="wkt")
            nc.sync.dma_start(wkt[:], wk_d[d * 128:(d + 1) * 128, :])
            nc.tensor.matmul(keys_ps[:], pt[:], wkt[:],
                             start=(d == 0), stop=(d == nch - 1))
        # normalize along free (two 64-wide aspect groups), fold w_s
        ksq = sb.tile([NL, S], F32, tag="ksq")
        ksc = sb.tile([NL, SDK], F32, tag="ksc")  # scratch square out
        for s in range(S):
            nc.scalar.activation(ksc[:, s * DK:(s + 1) * DK],
                                 keys_ps[:, s * DK:(s + 1) * DK],
                                 AF.Square,
                                 accum_out=ksq[:, s:s + 1])
        knorm = sb.tile([NL, S], F32, tag="knorm")
        nc.scalar.activation(knorm[:], ksq[:], AF.Sqrt)
        nc.vector.tensor_scalar_add(knorm[:], knorm[:], 1e-8)
        krec = sb.tile([NL, S], F32, tag="krec")
        nc.vector.reciprocal(krec[:], knorm[:])
        kn_w = sb.tile([NL, SDK], F32, tag="kn_w")
        for s, w_s in ((0, w0_f), (1, w1_f)):
            nc.vector.tensor_scalar(
                kn_w[:, s * DK:(s + 1) * DK],
                keys_ps[:, s * DK:(s + 1) * DK],
                krec[:, s:s + 1], float(w_s),
                op0=ALU.mult, op1=ALU.mult)

        if LEVEL <= 1:
            nc.sync.dma_start(out_d[:, :SDK], kn_w[:])
            return

        # ---------- phase B: queries ----------
        q_ps = psA.tile([BL, SDK], F32, tag="q")
        for c in range(4):
            nc.tensor.matmul(q_ps[:],
                             zt_sb[:, c * 128:(c + 1) * 128],
                             wq_sb[:, c * SDK:(c + 1) * SDK],
                             start=(c == 0), stop=(c == 3))
        qsq = sb.tile([BL, S], F32, tag="qsq")
        qsc = sb.tile([BL, SDK], F32, tag="qsc")
        for s in range(S):
            nc.scalar.activation(qsc[:, s * DK:(s + 1) * DK],
                                 q_ps[:, s * DK:(s + 1) * DK],
                                 AF.Square,
                                 accum_out=qsq[:, s:s + 1])
        qnorm = sb.tile([BL, S], F32, tag="qnorm")
        nc.scalar.activation(qnorm[:], qsq[:], AF.Sqrt)
        nc.vector.tensor_scalar_add(qnorm[:], qnorm[:], 1e-8)
        qrec = sb.tile([BL, S], F32, tag="qrec")
        nc.vector.reciprocal(qrec[:], qnorm[:])
        q_n = sb.tile([BL, SDK], F32, tag="q_n")
        for s in range(S):
            nc.vector.tensor_scalar(
                q_n[:, s * DK:(s + 1) * DK],
                q_ps[:, s * DK:(s + 1) * DK],
                qrec[:, s:s + 1], None, op0=ALU.mult)

    # ---------- AllGather keys ----------
    cc_in = dram.tile([NL, SDK], F32)
    cc_out = dram.tile([N, SDK], F32)
    nc.sync.dma_start(cc_in[:], kn_w[:])
    nc.gpsimd.collective_compute(
        "AllGather", ALU.bypass,
        replica_groups=[list(range(NCORES))],
        ins=[cc_in[:].opt()], outs=[cc_out[:].opt()],
    )

    if LEVEL <= 2:
        gat = sb.tile([NL, SDK], F32, tag="gat")
        nc.sync.dma_start(gat[:], cc_out[0:NL, :])
        nc.sync.dma_start(out_d[:, :SDK], gat[:])
        return

    with tc.tile_pool(name="psC", bufs=2, space="PSUM") as psC:
        # transpose q_n -> qnT [sdk, b]
        qn_tp = psC.tile([SDK, BL], F32, tag="ktr")
        nc.tensor.transpose(qn_tp[:], q_n[:], ident[:])
        qnT = sb.tile([SDK, BL], F32, tag="qnT")
        nc.scalar.activation(qnT[:], qn_tp[:], AF.Copy)

        # load gathered keys, transpose to knT [sdk, n]
        knT = sb.tile([SDK, N], F32, tag="knT")
        for c in range(NCORES):
            kc = sbs.tile([NL, SDK], F32, tag="kc")
            nc.sync.dma_start(kc[:], cc_out[c * NL:(c + 1) * NL, :])
            ktp = psC.tile([SDK, NL], F32, tag="ktr")
            nc.tensor.transpose(ktp[:], kc[:], ident[:])
            nc.scalar.activation(knT[:, c * NL:(c + 1) * NL],
                                 ktp[:], AF.Copy)

        # ---------- scores [b, n] ----------
        # sum over both aspects = single K=128 contraction (w folded in keys)
        scores = sb.tile([BL, N], F32, tag="scores")
        for j in range(2):
            sc_ps = psC.tile([BL, 512], F32, tag="sc")
            nc.tensor.matmul(sc_ps[:], qnT[:],
                             knT[:, j * 512:(j + 1) * 512],
                             start=True, stop=True)
            nc.scalar.activation(scores[:, j * 512:(j + 1) * 512],
                                 sc_ps[:], AF.Copy)

    if LEVEL <= 3:
        nc.sync.dma_start(out_d[:], scores[:, :D_MODEL])
        return

    # ---------- top-16 threshold + alpha ----------
    m8a = sb.tile([BL, 8], F32, tag="m8a")
    nc.vector.max(out=m8a[:], in_=scores[:])
    s_mr = sb.tile([BL, N], F32, tag="s_mr")
    nc.vector.match_replace(out=s_mr[:], in_to_replace=m8a[:],
                            in_values=scores[:], imm_value=-1e30)
    m8b = sb.tile([BL, 8], F32, tag="m8b")
    nc.vector.max(out=m8b[:], in_=s_mr[:])
    # threshold = 16th largest = m8b[:, 7]
    sig_b = sb.tile([BL, 1], F32, tag="sig_b")
    nc.vector.memset(sig_b[:], float(-LAMBDA_SHARP * tau_f))
    sig = sb.tile([BL, N], F32, tag="sig")
    nc.scalar.activation(sig[:], scores[:], AF.Sigmoid,
                         scale=LAMBDA_SHARP, bias=sig_b[:])
    ex = sb.tile([BL, N], F32, tag="ex")
    nc.scalar.activation(ex[:], scores[:], AF.Exp)
    ge = sb.tile([BL, N], F32, tag="ge")
    nc.vector.tensor_mul(ge[:], sig[:], ex[:])
    alpha = sb.tile([BL, N], F32, tag="alpha")
    den = sb.tile([BL, 1], F32, tag="den")
    nc.vector.scalar_tensor_tensor(
        out=alpha[:], in0=scores[:], scalar=m8b[:, 7:8], in1=ge[:],
        op0=ALU.is_ge, op1=ALU.mult, accum_out=den[:])
    nc.vector.tensor_scalar_add(den[:], den[:], 1e-8)
    rden = sb.tile([BL, 1], F32, tag="rden")
    nc.vector.reciprocal(rden[:], den[:])
    if DT_DYN != F32:
        alpha_dyn = sb.tile([BL, N], DT_DYN, tag="alpha_dyn")
        nc.vector.tensor_copy(alpha_dyn[:], alpha[:])
    else:
        alpha_dyn = alpha

    if LEVEL <= 4:
        nc.sync.dma_start(out_d[:], alpha[:, :D_MODEL])
        return

    s_sb = sb.tile([BL, NR], DT_DYN, tag="s_sb")

    with tc.tile_pool(name="psE", bufs=1, space="PSUM") as psE:
        # ---------- t = z @ V^T, s = alpha * t ----------
        for j in range(16):
            t_ps = psE.tile([BL, 512], F32, tag="t")
            for c in range(4):
                vtt = sbs.tile([128, 512], DT_DYN, tag="vtt")
                nc.sync.dma_start(
                    vtt[:],
                    vt_d[c * 128:(c + 1) * 128, j * 512:(j + 1) * 512])
                nc.tensor.matmul(t_ps[:],
                                 ztd_sb[:, c * 128:(c + 1) * 128],
                                 vtt[:], start=(c == 0), stop=(c == 3))
            nc.vector.tensor_tensor(
                out=s_sb[:, j * 512:(j + 1) * 512]
                    .rearrange("p (n r) -> p n r", r=R),
                in0=t_ps[:].rearrange("p (n r) -> p n r", r=R),
                in1=alpha_dyn[:, j * 64:(j + 1) * 64]
                    .unsqueeze(2).broadcast_to([BL, 64, R]),
                op=ALU.mult)

        if LEVEL <= 5:
            nc.sync.dma_start(out_d[:], s_sb[:, :D_MODEL])
            return

        # ---------- h1 = sT @ U (+ alphaT @ bias), unnormalized ----
        h1_ps = psE.tile([BL, D_MODEL], F32, tag="h1")
        for g in range(64):
            st_tp = psE.tile([128, 128], F32, tag="tr")
            nc.tensor.transpose(st_tp[:],
                                s_sb[:, g * 128:(g + 1) * 128],
                                ident[:])
            sT = sbs.tile([128, 128], DT_DYN, tag="sT")
            nc.scalar.activation(sT[:], st_tp[:], AF.Copy)
            upt = sbs.tile([128, D_MODEL], DT_DYN, tag="upt")
            nc.sync.dma_start(upt[:], up_d[g * 128:(g + 1) * 128, :])
            nc.tensor.matmul(h1_ps[:], sT[:], upt[:],
                             start=(g == 0), stop=(LEVEL <= 6 and g == 63))
        if LEVEL >= 7:
            for c in range(NCORES):
                al_tp = psE.tile([128, 128], F32, tag="tr")
                nc.tensor.transpose(al_tp[:],
                                    alpha[:, c * 128:(c + 1) * 128],
                                    ident[:])
                aT = sbs.tile([128, 128], DT_DYN, tag="aT")
                nc.scalar.activation(aT[:], al_tp[:], AF.Copy)
                pbt = sbs.tile([128, D_MODEL], DT_DYN, tag="pbt")
                nc.sync.dma_start(pbt[:], pb_d[c * NL:(c + 1) * NL, :])
                nc.tensor.matmul(h1_ps[:], aT[:], pbt[:],
                                 start=False, stop=(c == NCORES - 1))

        # ---------- h2 = z @ W_base^T ----------
        h2_ps = psE.tile([BL, D_MODEL], F32, tag="h2")
        for c in range(4):
            wbt = sbs.tile([128, D_MODEL], F32, tag="wbt")
            nc.sync.dma_start(wbt[:], wbt_d[c * 128:(c + 1) * 128, :])
            nc.tensor.matmul(h2_ps[:],
                             zt_sb[:, c * 128:(c + 1) * 128],
                             wbt[:], start=(c == 0), stop=(c == 3))

        # ---------- combine ----------
        A_sb = sb.tile([BL, D_MODEL], F32, tag="A")
        nc.vector.tensor_scalar(A_sb[:], h1_ps[:], rden[:], None,
                                op0=ALU.mult)
        nc.vector.tensor_add(A_sb[:], A_sb[:], h2_ps[:])

    # ---------- layernorm ----------
    x_sb = sb.tile([BL, D_MODEL], F32, tag="x")
    nc.vector.scalar_tensor_tensor(
        out=x_sb[:], in0=A_sb[:], scalar=float(gamma_f), in1=zb_sb[:],
        op0=ALU.mult, op1=ALU.add)
    mean = sb.tile([BL, 1], F32, tag="mean")
    nc.vector.reduce_sum(mean[:], x_sb[:], axis=mybir.AxisListType.X)
    nc.vector.tensor_scalar_mul(mean[:], mean[:], 1.0 / D_MODEL)
    xc = sb.tile([BL, D_MODEL], F32, tag="xc")
    nc.vector.tensor_scalar(xc[:], x_sb[:], mean[:], None,
                            op0=ALU.subtract)
    xsq = sb.tile([BL, D_MODEL], F32, tag="xsq")
    ssq = sb.tile([BL, 1], F32, tag="ssq")
    nc.scalar.activation(xsq[:], xc[:], AF.Square, accum_out=ssq[:])
    vare = sb.tile([BL, 1], F32, tag="vare")
    nc.vector.tensor_scalar(vare[:], ssq[:], 1.0 / D_MODEL, LN_EPS,
                            op0=ALU.mult, op1=ALU.add)
    sd = sb.tile([BL, 1], F32, tag="sd")
    nc.scalar.activation(sd[:], vare[:], AF.Sqrt)
    rstd = sb.tile([BL, 1], F32, tag="rstd")
    nc.vector.reciprocal(rstd[:], sd[:])
    y1 = sb.tile([BL, D_MODEL], F32, tag="y1")
    nc.vector.tensor_scalar(y1[:], xc[:], rstd[:], None, op0=ALU.mult)
    out_sb = sb.tile([BL, D_MODEL], F32, tag="out_sb")
    nc.vector.tensor_mul(out_sb[:], y1[:], ls_sb[:])
    nc.vector.tensor_add(out_sb[:], out_sb[:], lb_sb[:])
    nc.sync.dma_start(out_d[:], out_sb[:])


def kernel(z, pool_vectors, W_Q, W_K, aspect_logits, tau,
           W_base, b_base, gamma, ln_scale, ln_bias):
    global LAST_EXEC_NS
    z = np.asarray(z, np.float32)
    pool = np.asarray(pool_vectors, np.float32)
    W_Q = np.asarray(W_Q, np.float32)
    W_K = np.asarray(W_K, np.float32)
    aspect_logits = np.asarray(aspect_logits, np.float32)
    tau_f = float(np.asarray(tau))
    W_base = np.asarray(W_base, np.float32)
    b_base = np.asarray(b_base, np.float32)
    gamma_f = float(np.asarray(gamma))
    ln_scale = np.asarray(ln_scale, np.float32)
    ln_bias = np.asarray(ln_bias, np.float32)

    e = np.exp(aspect_logits - aspect_logits.max())
    w = e / e.sum()
    w0_f, w1_f = float(w[0]), float(w[1])

    nc = _build(tau_f, w0_f, w1_f, gamma_f)

    # ---- host-side layout prep ----
    np_keys = _np_dt(DT_KEYS)
    np_dyn = _np_dt(DT_DYN)
    wk_cat = np.concatenate([W_K[0], W_K[1]], axis=1).astype(np_keys)  # [D, 128]
    wq_cat = np.concatenate([W_Q[0], W_Q[1]], axis=1).astype(np.float32)
    # V^T: [e, n*R + r] = pool[n, 4096 + r*512 + e]
    vt = np.ascontiguousarray(
        pool[:, U_END:V_END].reshape(N, R, D_MODEL).transpose(2, 0, 1)
        .reshape(D_MODEL, NR)).astype(np_dyn)
    # U_perm: [n*R + r, c] = pool[n, c*R + r]
    up = np.ascontiguousarray(
        pool[:, :U_END].reshape(N, D_MODEL, R).transpose(0, 2, 1)
        .reshape(NR, D_MODEL)).astype(np_dyn)
    pb = np.ascontiguousarray(pool[:, V_END:B_END]).astype(np_dyn)  # [N, 512]
    wbt = np.ascontiguousarray(W_base.T)  # [a, c]
    gb = np.broadcast_to((gamma_f * b_base).astype(np.float32),
                         (BL, D_MODEL)).copy()
    ls = np.broadcast_to(ln_scale, (BL, D_MODEL)).copy()
    lb = np.broadcast_to(ln_bias, (BL, D_MODEL)).copy()

    in_maps = []
    for c in range(NCORES):
        z_loc = np.ascontiguousarray(z[c * BL:(c + 1) * BL])
        zt_loc = np.ascontiguousarray(z_loc.T)
        poolT_loc = np.ascontiguousarray(
            pool[c * NL:(c + 1) * NL].T).astype(np_keys)
        in_maps.append({
            "z": z_loc, "zt": zt_loc, "poolT": poolT_loc,
            "wk": wk_cat, "wq": wq_cat, "vt": vt, "up": up, "pb": pb,
            "wbt": wbt, "gb": gb, "ls": ls, "lb": lb,
        })

    res = run_bass_kernel_spmd(nc, in_maps, core_ids=list(range(NCORES)),
                               trace=TRACE)
    LAST_EXEC_NS = res.exec_time_ns
    out = np.concatenate([res.results[c]["out"] for c in range(NCORES)],
                         axis=0)
    return out.astype(np.float32)
